# revision 11
# baseline (speedup 1.0000x reference)
"""Trainium2 Bass kernel for nn_LiquidAIModel_47193100648724.

3-layer MoE (shared top-2 routing from token 0) -> 2-layer transformer
encoder -> 2-layer decoder (self+cross attention) -> 2 linear heads.

Strategy: sequence-parallel over 8 NeuronCores (256 tokens/core, plus a
replicated token-0 column for routing). Activations are kept
feature-major (x^T layout: features on partitions, tokens on the free
dim) end-to-end, which makes every matmul/LN/softmax transpose-free.
Matmuls run in bf16 with fp32 PSUM accumulation; routing is computed in
exact fp32 via host-precomputed weight-product matrices contracted with
token 0 on device. K/V for each attention block are all-gathered across
cores (DRAM bounce + collective).
"""

import os
import sys

sys.path.insert(0, "/opt/trn_rl_repo")

import numpy as np
import ml_dtypes

import concourse.bass as bass
import concourse.bacc as bacc
import concourse.mybir as mybir
import concourse.tile as tile
from concourse.bass_utils import run_bass_kernel_spmd

f32 = mybir.dt.float32
bf16 = mybir.dt.bfloat16
AF = mybir.ActivationFunctionType
ALU = mybir.AluOpType
AX = mybir.AxisListType

P = 128
CORES = 8
S, D, H, OUT, E, NHEAD, FF = 2048, 1024, 1024, 1024, 4, 4, 2048
DH = H // NHEAD            # 256
SC = S // CORES            # 256 tokens per core
NT = SC + 1                # 257: col 0 = token-0 replica
KS = H // P                # 8 k-subtiles for 1024-dim
EPS = 1e-5
NB16 = np.dtype(ml_dtypes.bfloat16)

DEBUG = bool(int(os.environ.get("KBENCH_DEBUG", "0")))


def _bf(a):
    return np.ascontiguousarray(np.asarray(a, np.float32)).astype(NB16)


def _f32(a):
    return np.ascontiguousarray(np.asarray(a, np.float32))


def _ppack(b):
    """[n*128] fp32 -> per-partition pack [128, n] (b[mt*128+p] -> [p, mt])."""
    b = _f32(b)
    n = b.shape[0] // P
    return np.ascontiguousarray(b.reshape(n, P).T)


def _prep_gmat(p):
    """Host-side routing precompute (fp64): G_all [108, 1152] such that
    P = G_all @ [x0; 1; 0...] gives every inner product needed to form all
    three gate-logit vectors on device in exact fp32."""
    m = [np.asarray(x, np.float64) for x in []]  # noqa
    wg = [np.asarray(p['moe'][l]['wg'], np.float64) for l in range(3)]
    bg = [np.asarray(p['moe'][l]['bg'], np.float64) for l in range(3)]
    We = [np.asarray(p['moe'][l]['We'], np.float64) for l in range(3)]
    be = [np.asarray(p['moe'][l]['be'], np.float64) for l in range(3)]
    G = np.zeros((108, 1152), np.float64)
    G[0:4, :D] = wg[0]
    G[0:4, D] = bg[0]
    for e in range(E):
        A = wg[1] @ We[0][e]               # [4, D]
        G[4 + 4 * e: 8 + 4 * e, :D] = A
        G[4 + 4 * e: 8 + 4 * e, D] = wg[1] @ be[0][e]
    G[20:24, D] = bg[1]
    for e in range(E):
        WE = wg[2] @ We[1][e]              # [4, H]
        for f in range(E):
            r0 = 24 + 4 * (4 * e + f)
            G[r0:r0 + 4, :D] = WE @ We[0][f]
            G[r0:r0 + 4, D] = WE @ be[0][f]
    for e in range(E):
        G[88 + 4 * e: 92 + 4 * e, D] = wg[2] @ be[1][e]
    G[104:108, D] = bg[2]
    # device layout [128, 9, 108]: gmat[p, k, r] = G[r, k*128+p]
    gm = G.T.reshape(9, P, 108).transpose(1, 0, 2)
    return _f32(gm)


def _prep_inputs(x, p):
    """Build the per-core input maps."""
    x = np.asarray(x, np.float32)
    shared = {}
    shared['gmat'] = _prep_gmat(p)
    x0p = np.zeros(1152, np.float32)
    x0p[:D] = x[0]
    x0p[D] = 1.0
    shared['x0f'] = _f32(x0p.reshape(9, P).T.reshape(P, 9, 1))

    for l in range(3):
        We = np.asarray(p['moe'][l]['We'], np.float32)       # [E, H, D]
        wstack = We.transpose(0, 2, 1).reshape(E * D, H)     # [4096, 1024]
        shared[f'moe{l}_w'] = _bf(wstack)
        be = np.asarray(p['moe'][l]['be'], np.float32)       # [E, H]
        # [128, 4, 8]: [p, e, mt] = be[e, mt*128+p]
        shared[f'moe{l}_b'] = _f32(be.reshape(E, KS, P).transpose(2, 0, 1))

    def attn_pack(prefix, a):
        shared[f'{prefix}_wq'] = _bf(np.asarray(a['wq'], np.float32).T)
        shared[f'{prefix}_wk'] = _bf(np.asarray(a['wk'], np.float32).T)
        shared[f'{prefix}_wv'] = _bf(np.asarray(a['wv'], np.float32).T)
        shared[f'{prefix}_wo'] = _bf(np.asarray(a['wo'], np.float32).T)
        shared[f'{prefix}_bq'] = _ppack(a['bq'])
        shared[f'{prefix}_bk'] = _ppack(a['bk'])
        shared[f'{prefix}_bo'] = _ppack(a['bo'])
        shared[f'{prefix}_bvb'] = _bf(np.broadcast_to(np.asarray(a['bv'], np.float32), (P, H)))

    def ln_pack(prefix, ln):
        shared[f'{prefix}_g'] = _ppack(ln['g'])
        shared[f'{prefix}_b'] = _ppack(ln['b'])

    def ffn_pack(prefix, lp):
        shared[f'{prefix}_ff1'] = _bf(np.asarray(lp['ff1']['w'], np.float32).T)   # [1024, 2048]
        shared[f'{prefix}_ff1b'] = _ppack(lp['ff1']['b'])                          # [128, 16]
        shared[f'{prefix}_ff2'] = _bf(np.asarray(lp['ff2']['w'], np.float32).T)   # [2048, 1024]
        shared[f'{prefix}_ff2b'] = _ppack(lp['ff2']['b'])

    for i, lp in enumerate(p['enc']):
        attn_pack(f'enc{i}_sa', lp['sa'])
        ln_pack(f'enc{i}_ln1', lp['ln1'])
        ffn_pack(f'enc{i}', lp)
        ln_pack(f'enc{i}_ln2', lp['ln2'])
    ln_pack('enc_norm', p['enc_norm'])
    for i, lp in enumerate(p['dec']):
        attn_pack(f'dec{i}_sa', lp['sa'])
        ln_pack(f'dec{i}_ln1', lp['ln1'])
        attn_pack(f'dec{i}_ca', lp['ca'])
        ln_pack(f'dec{i}_ln2', lp['ln2'])
        ffn_pack(f'dec{i}', lp)
        ln_pack(f'dec{i}_ln3', lp['ln3'])
    ln_pack('dec_norm', p['dec_norm'])
    shared['fc_w'] = _bf(np.asarray(p['fc']['w'], np.float32).T)
    shared['fc_b'] = _ppack(p['fc']['b'])
    shared['out_w'] = _bf(np.asarray(p['out']['w'], np.float32).T)
    shared['out_b'] = _ppack(p['out']['b'])

    in_maps = []
    for c in range(CORES):
        m = dict(shared)
        xc = np.empty((NT, D), np.float32)
        xc[0] = x[0]
        xc[1:] = x[c * SC:(c + 1) * SC]
        m['xT'] = _bf(xc.T)          # [1024, 257] bf16
        in_maps.append(m)
    return in_maps


# ---------------------------------------------------------------------------
# device kernel builder
# ---------------------------------------------------------------------------

def build(nc):
    dt_in = {}

    def din(name, shape, dt):
        dt_in[name] = nc.dram_tensor(name, list(shape), dt, kind="ExternalInput").ap()
        return dt_in[name]

    din('xT', (D, NT), bf16)
    din('gmat', (P, 9, 108), f32)
    din('x0f', (P, 9, 1), f32)
    for l in range(3):
        din(f'moe{l}_w', (E * D, H), bf16)
        din(f'moe{l}_b', (P, E, KS), f32)

    def attn_decl(prefix):
        for w in ('wq', 'wk', 'wv', 'wo'):
            din(f'{prefix}_{w}', (H, H), bf16)
        for b in ('bq', 'bk', 'bo'):
            din(f'{prefix}_{b}', (P, KS), f32)
        din(f'{prefix}_bvb', (P, H), bf16)

    def ln_decl(prefix):
        din(f'{prefix}_g', (P, KS), f32)
        din(f'{prefix}_b', (P, KS), f32)

    def ffn_decl(prefix):
        din(f'{prefix}_ff1', (H, FF), bf16)
        din(f'{prefix}_ff1b', (P, FF // P), f32)
        din(f'{prefix}_ff2', (FF, H), bf16)
        din(f'{prefix}_ff2b', (P, KS), f32)

    for i in range(2):
        attn_decl(f'enc{i}_sa'); ln_decl(f'enc{i}_ln1')
        ffn_decl(f'enc{i}'); ln_decl(f'enc{i}_ln2')
    ln_decl('enc_norm')
    for i in range(2):
        attn_decl(f'dec{i}_sa'); ln_decl(f'dec{i}_ln1')
        attn_decl(f'dec{i}_ca'); ln_decl(f'dec{i}_ln2')
        ffn_decl(f'dec{i}'); ln_decl(f'dec{i}_ln3')
    ln_decl('dec_norm')
    din('fc_w', (H, H), bf16)
    din('fc_b', (P, KS), f32)
    din('out_w', (H, OUT), bf16)
    din('out_b', (P, KS), f32)

    out_d = nc.dram_tensor("out", [OUT, SC], f32, kind="ExternalOutput").ap()
    dbg = {}
    if DEBUG:
        for name in ('dbg_h', 'dbg_mem'):
            dbg[name] = nc.dram_tensor(name, [H, NT], f32, kind="ExternalOutput").ap()

    with tile.TileContext(nc) as tc:
        _body(nc, tc, dt_in, out_d, dbg)
    return dt_in


def _body(nc, tc, din, out_d, dbg):
    import contextlib
    ctx = contextlib.ExitStack()
    with ctx:
        cpool = ctx.enter_context(tc.tile_pool(name="const", bufs=1))
        hf = ctx.enter_context(tc.tile_pool(name="hf", bufs=3))
        hb = ctx.enter_context(tc.tile_pool(name="hb", bufs=3))
        wa = ctx.enter_context(tc.tile_pool(name="wa", bufs=2))
        xsp = ctx.enter_context(tc.tile_pool(name="xsp", bufs=1))
        ffp = ctx.enter_context(tc.tile_pool(name="ffp", bufs=1))
        kvp = ctx.enter_context(tc.tile_pool(name="kvp", bufs=1))
        khp = ctx.enter_context(tc.tile_pool(name="khp", bufs=2))
        ahp = ctx.enter_context(tc.tile_pool(name="ahp", bufs=1))
        stp = ctx.enter_context(tc.tile_pool(name="stp", bufs=1))
        qop = ctx.enter_context(tc.tile_pool(name="qop", bufs=1))
        lnp = ctx.enter_context(tc.tile_pool(name="lnp", bufs=1))
        smp = ctx.enter_context(tc.tile_pool(name="smp", bufs=2))
        gp = ctx.enter_context(tc.tile_pool(name="gp", bufs=2))
        pp = ctx.enter_context(tc.tile_pool(name="pp", bufs=8, space="PSUM"))
        dram = ctx.enter_context(tc.tile_pool(name="dram", bufs=2, space="DRAM"))
        wm = wa

        # ---- constants ----
        ones_b = cpool.tile([P, 1], bf16, tag="ones_b")
        nc.vector.memset(ones_b[:], 1.0)
        ones_f = cpool.tile([1, P], f32, tag="ones_f")
        nc.vector.memset(ones_f[:], 1.0)

        def load_const(name, shape, dt=f32):
            t = cpool.tile(list(shape), dt, tag=name)
            nc.sync.dma_start(t[:], din[name][:])
            return t

        gmat = load_const('gmat', (P, 9, 108))
        x0f = load_const('x0f', (P, 9, 1))

        # weight loader: [Kdim, Mdim] DRAM -> [128, Kdim/128, Mdim] SBUF bf16
        def load_w(pool, name, kdim, mdim, tag):
            t = pool.tile([P, kdim // P, mdim], bf16, tag=tag)
            nc.sync.dma_start(t[:], din[name].rearrange("(ks p) m -> p ks m", p=P))
            return t

        def small(pool, name, shape=(P, KS), tag=None, dt=f32):
            t = pool.tile(list(shape), dt, tag=tag or name[-12:])
            nc.sync.dma_start(t[:], din[name][:])
            return t

        # ---------------- generic projection ----------------
        def proj(w_sb, x_bf, n_mt, n_ks, consumer, cols=None):
            """out[mt] = sum_ks w_sb[:, ks, mt*128:...].T @ x_bf[:, ks, cols]"""
            for mt in range(n_mt):
                pt = pp.tile([P, NT], f32, tag="pp")
                ncol = NT if cols is None else cols[1] - cols[0]
                for ks in range(n_ks):
                    rhs = x_bf[:, ks, :] if cols is None else x_bf[:, ks, cols[0]:cols[1]]
                    nc.tensor.matmul(pt[:, :ncol], w_sb[:, ks, mt * P:(mt + 1) * P], rhs,
                                     start=(ks == 0), stop=(ks == n_ks - 1))
                consumer(mt, pt[:, :ncol])

        # ---------------- gate helpers ----------------
        def gate_from_logits(l4):
            """l4 [1,4] fp32 logits -> w [1,4] fp32: softmax then top-2 mask."""
            el = gp.tile([1, 4], f32, tag="g_el")
            nc.scalar.activation(el[:], l4[:], AF.Exp)
            sm = gp.tile([1, 1], f32, tag="g_s")
            nc.vector.reduce_sum(sm[:], el[:], axis=AX.X)
            rc = gp.tile([1, 1], f32, tag="g_r")
            nc.vector.reciprocal(rc[:], sm[:])
            g = gp.tile([1, 4], f32, tag="g_g")
            nc.vector.tensor_scalar_mul(g[:], el[:], rc[:])
            m1 = gp.tile([1, 1], f32, tag="g_m1")
            nc.vector.reduce_max(m1[:], g[:], axis=AX.X)
            eq = gp.tile([1, 4], f32, tag="g_eq")
            nc.vector.tensor_tensor(eq[:], g[:], m1.to_broadcast((1, 4)), ALU.is_equal)
            gm = gp.tile([1, 4], f32, tag="g_gm")
            nc.vector.scalar_tensor_tensor(gm[:], eq[:], -1e9, g[:], ALU.mult, ALU.add)
            m2 = gp.tile([1, 1], f32, tag="g_m2")
            nc.vector.reduce_max(m2[:], gm[:], axis=AX.X)
            msk = gp.tile([1, 4], f32, tag="g_msk")
            nc.vector.tensor_tensor(msk[:], g[:], m2.to_broadcast((1, 4)), ALU.is_ge)
            w = gp.tile([1, 4], f32, tag="g_w")
            nc.vector.tensor_tensor(w[:], g[:], msk[:], ALU.mult)
            return w

        def bcast_w(w4):
            """[1,4] fp32 -> [128,4] fp32 via PE."""
            pb = pp.tile([P, 4], f32, tag="pp")
            nc.tensor.matmul(pb[:], ones_f[:], w4[:], start=True, stop=True)
            wb = gp.tile([P, 4], f32, tag="g_wb")
            nc.scalar.copy(wb[:], pb[:])
            return wb

        # ---------------- MoE stack ----------------
        # P-vector: every inner product of [x0;1] with precomputed matrices
        pg = pp.tile([1, 108], f32, tag="pp")
        for k in range(9):
            nc.tensor.matmul(pg[:], x0f[:, k, :], gmat[:, k, :],
                             start=(k == 0), stop=(k == 8))
        Pv = gp.tile([1, 108], f32, tag="g_P")
        nc.scalar.copy(Pv[:], pg[:])

        xT = hb.tile([P, KS, NT], bf16, tag="hb")
        nc.sync.dma_start(xT[:], din['xT'].rearrange("(ks p) t -> p ks t", p=P))

        cur_b = xT      # bf16 input to current MoE layer
        cur_f = None
        w_prev = []     # routing weight vectors [1,4] per layer

        for l in range(3):
            # --- logits for this layer's gate (exact fp32 from P) ---
            if l == 0:
                l4 = gp.tile([1, 4], f32, tag="g_l4")
                nc.vector.tensor_copy(out=l4[:], in_=Pv[:, 0:4])
            elif l == 1:
                l4 = gp.tile([1, 4], f32, tag="g_l4")
                w0 = w_prev[0]
                nc.vector.tensor_scalar_mul(l4[:], Pv[:, 4:8], w0[:, 0:1])
                for e in range(1, E):
                    nc.vector.scalar_tensor_tensor(
                        l4[:], Pv[:, 4 + 4 * e:8 + 4 * e], w0[:, e:e + 1], l4[:],
                        ALU.mult, ALU.add)
                nc.vector.tensor_tensor(l4[:], l4[:], Pv[:, 20:24], ALU.add)
            else:
                w0, w1 = w_prev
                q = gp.tile([1, 4, 4], f32, tag="g_q")   # q[e,f] = w1[e]*w0[f]
                nc.vector.tensor_tensor(
                    q[:], w0[:, None, :].to_broadcast((1, 4, 4)),
                    w1[:, :, None].to_broadcast((1, 4, 4)), ALU.mult)
                qf = q.rearrange("a b c -> a (b c)")
                l4 = gp.tile([1, 4], f32, tag="g_l4")
                nc.vector.tensor_scalar_mul(l4[:], Pv[:, 24:28], qf[:, 0:1])
                for i in range(1, 16):
                    nc.vector.scalar_tensor_tensor(
                        l4[:], Pv[:, 24 + 4 * i:28 + 4 * i], qf[:, i:i + 1], l4[:],
                        ALU.mult, ALU.add)
                for e in range(E):
                    nc.vector.scalar_tensor_tensor(
                        l4[:], Pv[:, 88 + 4 * e:92 + 4 * e], w1[:, e:e + 1], l4[:],
                        ALU.mult, ALU.add)
                nc.vector.tensor_tensor(l4[:], l4[:], Pv[:, 104:108], ALU.add)

            wv4 = gate_from_logits(l4)
            w_prev.append(wv4)
            wbc = bcast_w(wv4)

            # --- effective bias: beff[:, mt] = sum_e w_e * be[p, e, mt] ---
            moeb = small(gp, f'moe{l}_b', (P, E, KS), tag="g_moeb")
            beff = gp.tile([P, KS], f32, tag="g_beff")
            nc.vector.tensor_scalar_mul(beff[:], moeb[:, 0, :], wbc[:, 0:1])
            for e in range(1, E):
                nc.vector.scalar_tensor_tensor(beff[:], moeb[:, e, :], wbc[:, e:e + 1],
                                               beff[:], ALU.mult, ALU.add)

            # --- X' = stack_e(w_e * x) [128, 32, NT] bf16 ---
            xs = xsp.tile([P, 4 * KS, NT], bf16, tag="xs")
            for e in range(E):
                nc.vector.tensor_scalar_mul(xs[:, KS * e:KS * (e + 1), :], cur_b[:],
                                            wbc[:, e:e + 1])

            # --- big matmul: h = Wstack^T-stacked contraction, K = 4096 ---
            # load as 4 chunks of [128, 8, 1024] streamed through the shared
            # weight slots (each chunk is consumed in one ck-pass)
            chunks = []
            for ck in range(4):
                t = wm.tile([P, KS, H], bf16, tag="wa")
                nc.sync.dma_start(
                    t[:], din[f'moe{l}_w'][ck * D:(ck + 1) * D, :]
                    .rearrange("(ks p) m -> p ks m", p=P))
                chunks.append(t)

            nh_f = hf.tile([P, KS, NT], f32, tag="hf")
            nh_b = hb.tile([P, KS, NT], bf16, tag="hb")
            psums = [pp.tile([P, NT], f32, tag="pp", name=f"psum_moe{l}_{i}") for i in range(KS)]
            for ck in range(4):
                for ksw in range(KS):
                    for mt in range(KS):
                        nc.tensor.matmul(
                            psums[mt][:], chunks[ck][:, ksw, mt * P:(mt + 1) * P],
                            xs[:, ck * KS + ksw, :],
                            start=(ck == 0 and ksw == 0),
                            stop=(ck == 3 and ksw == KS - 1))
            for mt in range(KS):
                nc.scalar.activation(nh_f[:, mt, :], psums[mt][:], AF.Identity,
                                     bias=beff[:, mt:mt + 1])
                nc.vector.tensor_copy(out=nh_b[:, mt, :], in_=nh_f[:, mt, :])
            cur_f, cur_b = nh_f, nh_b

        if DEBUG:
            nc.sync.dma_start(dbg['dbg_h'].rearrange("(ks p) t -> p ks t", p=P), cur_f[:])

        # ---------------- layernorm ----------------
        def layernorm(r_f, prefix):
            """r_f [128,8,NT] fp32 -> (y_f32, y_bf16)."""
            g_t = small(smp, f'{prefix}_g', tag="ln_g")
            b_t = small(smp, f'{prefix}_b', tag="ln_b")
            rb = lnp.tile([P, KS, NT], bf16, tag="ln_rb")
            nc.vector.tensor_copy(out=rb[:], in_=r_f[:])
            sq = lnp.tile([P, KS, NT], bf16, tag="ln_sq")
            nc.scalar.activation(sq[:], r_f[:], AF.Square)
            psum_s = pp.tile([1, NT], f32, tag="pp")
            psum_q = pp.tile([1, NT], f32, tag="pp")
            for ks in range(KS):
                nc.tensor.matmul(psum_s[:], ones_b[:], rb[:, ks, :],
                                 start=(ks == 0), stop=(ks == KS - 1))
            for ks in range(KS):
                nc.tensor.matmul(psum_q[:], ones_b[:], sq[:, ks, :],
                                 start=(ks == 0), stop=(ks == KS - 1))
            m = stp.tile([1, NT], f32, tag="ln_m")
            nc.vector.tensor_scalar_mul(m[:], psum_s[:], 1.0 / H)
            msq = stp.tile([1, NT], f32, tag="ln_msq")
            nc.vector.tensor_scalar_mul(msq[:], psum_q[:], 1.0 / H)
            mm = stp.tile([1, NT], f32, tag="ln_mm")
            nc.vector.tensor_tensor(mm[:], m[:], m[:], ALU.mult)
            vpe = stp.tile([1, NT], f32, tag="ln_vpe")
            nc.vector.scalar_tensor_tensor(vpe[:], mm[:], -1.0, msq[:], ALU.mult, ALU.add)
            nc.vector.tensor_scalar_add(vpe[:], vpe[:], EPS)
            # rstd = exp(-0.5*ln(vpe)), one Newton step
            lnv = stp.tile([1, NT], f32, tag="ln_lnv")
            nc.scalar.activation(lnv[:], vpe[:], AF.Ln)
            r0 = stp.tile([1, NT], f32, tag="ln_r0")
            nc.scalar.activation(r0[:], lnv[:], AF.Exp, scale=-0.5)
            t = stp.tile([1, NT], f32, tag="ln_t")
            nc.vector.tensor_tensor(t[:], r0[:], r0[:], ALU.mult)
            nc.vector.tensor_tensor(t[:], t[:], vpe[:], ALU.mult)
            nc.vector.tensor_scalar(t[:], t[:], -0.5, 1.5, ALU.mult, ALU.add)
            rstd = stp.tile([1, NT], f32, tag="ln_rstd")
            nc.vector.tensor_tensor(rstd[:], r0[:], t[:], ALU.mult)
            nmr = stp.tile([1, NT], f32, tag="ln_nmr")   # -m*rstd
            nc.vector.tensor_tensor(nmr[:], m[:], rstd[:], ALU.mult)
            nc.vector.tensor_scalar_mul(nmr[:], nmr[:], -1.0)
            # broadcast rstd and -m*rstd across partitions via PE
            pa = pp.tile([P, NT], f32, tag="pp")
            nc.tensor.matmul(pa[:], ones_f[:], rstd[:], start=True, stop=True)
            pc = pp.tile([P, NT], f32, tag="pp")
            nc.tensor.matmul(pc[:], ones_f[:], nmr[:], start=True, stop=True)
            y_f = hf.tile([P, KS, NT], f32, tag="hf")
            y_b = hb.tile([P, KS, NT], bf16, tag="hb")
            for mt in range(KS):
                u = smp.tile([P, NT], f32, tag="ln_u")
                nc.vector.tensor_tensor(u[:], r_f[:, mt, :], pa[:], ALU.mult)
                nc.vector.tensor_tensor(u[:], u[:], pc[:], ALU.add)
                nc.scalar.activation(y_f[:, mt, :], u[:], AF.Identity,
                                     bias=b_t[:, mt:mt + 1], scale=g_t[:, mt:mt + 1])
                nc.vector.tensor_copy(out=y_b[:, mt, :], in_=y_f[:, mt, :])
            return y_f, y_b

        # ---------------- attention ----------------
        gather_seq = [0]

        def kv_local(src_b, prefix):
            """K_loc [128,8,SC] bf16, V_loc [128,2,H] bf16 from cols 1..NT."""
            wk = load_w(wa, f'{prefix}_wk', H, H, tag="wa")
            bk = small(smp, f'{prefix}_bk', tag="bk")
            k_loc = kvp.tile([P, KS, SC], bf16, tag="kloc")
            def k_cons(mt, pt):
                nc.scalar.activation(k_loc[:, mt, :], pt, AF.Identity,
                                     bias=bk[:, mt:mt + 1])
            proj(wk, src_b, KS, KS, k_cons, cols=(1, NT))
            wv = load_w(wa, f'{prefix}_wv', H, H, tag="wa")
            bvb = smp.tile([P, H], bf16, tag="bvb")
            nc.sync.dma_start(bvb[:], din[f'{prefix}_bvb'][:])
            v_loc = kvp.tile([P, 2, H], bf16, tag="vloc")
            for tt in range(2):
                for nh2 in range(2):
                    pt = pp.tile([P, 512], f32, tag="pp")
                    for ks in range(KS):
                        nc.tensor.matmul(pt[:], src_b[:, ks, 1 + tt * P:1 + (tt + 1) * P],
                                         wv[:, ks, nh2 * 512:(nh2 + 1) * 512],
                                         start=(ks == 0), stop=(ks == KS - 1))
                    nc.vector.tensor_tensor(v_loc[:, tt, nh2 * 512:(nh2 + 1) * 512],
                                            pt[:], bvb[:, nh2 * 512:(nh2 + 1) * 512],
                                            ALU.add)
            return k_loc, v_loc

        def kv_gather(pairs):
            """pairs: list of (k_loc, v_loc). Returns gathered DRAM tile
            [CORES, 2*len(pairs), 128, 2048]."""
            n = 2 * len(pairs)
            kv_in = dram.tile([n, P, SC * KS], bf16, tag=f"kvin{n}")
            for i, (k_loc, v_loc) in enumerate(pairs):
                nc.sync.dma_start(kv_in[2 * i], k_loc.rearrange("p a b -> p (a b)"))
                nc.sync.dma_start(kv_in[2 * i + 1], v_loc.rearrange("p a b -> p (a b)"))
            kv_out = dram.tile([CORES, n, P, SC * KS], bf16, tag=f"kvout{n}")
            nc.gpsimd.collective_compute(
                "AllGather", ALU.bypass,
                replica_groups=[list(range(CORES))],
                ins=[kv_in.opt()], outs=[kv_out.opt()])
            return kv_out

        def attention(src_f, src_b, prefix, kv_out, slot):
            """Full MHA block. Returns r = src_f + attention(...) fp32 [128,8,NT].
            kv_out: gathered dram tile, slot: index of (K,V) pair in it."""
            wq = load_w(wa, f'{prefix}_wq', H, H, tag="wa")
            bq = small(smp, f'{prefix}_bq', tag="bq")
            q_bf = qop.tile([P, KS, NT], bf16, tag="qbf")
            def q_cons(mt, pt):
                nc.scalar.activation(q_bf[:, mt, :], pt, AF.Identity,
                                     bias=bq[:, mt:mt + 1])
            proj(wq, src_b, KS, KS, q_cons)

            o_sb = qop.tile([P, KS, NT], bf16, tag="osb")
            for h in range(NHEAD):
                k_h = khp.tile([P, 2, S], bf16, tag="kh")
                for d in range(2):
                    nc.sync.dma_start(
                        k_h[:, d, :].rearrange("p (c t) -> p c t", c=CORES),
                        kv_out[:, 2 * slot, :, (2 * h + d) * SC:(2 * h + d + 1) * SC]
                        .rearrange("c p t -> p c t"))
                v_h = kvp.tile([P, 2 * CORES, DH], bf16, tag="vh")
                for tt in range(2):
                    nc.sync.dma_start(
                        v_h.rearrange("p (c tt) f -> p c tt f", c=CORES)[:, :, tt, :],
                        kv_out[:, 2 * slot + 1, :, tt * H + h * DH: tt * H + (h + 1) * DH]
                        .rearrange("c p f -> p c f"))

                a_h = ahp.tile([P, 2 * CORES, NT], bf16, tag="ah")
                for kt in range(2 * CORES):
                    ps = pp.tile([P, NT], f32, tag="pp")
                    nc.tensor.matmul(ps[:], k_h[:, 0, kt * P:(kt + 1) * P],
                                     q_bf[:, 2 * h, :], start=True, stop=False)
                    nc.tensor.matmul(ps[:], k_h[:, 1, kt * P:(kt + 1) * P],
                                     q_bf[:, 2 * h + 1, :], start=False, stop=True)
                    nc.scalar.activation(a_h[:, kt, :], ps[:], AF.Exp, scale=1.0 / 16)

                po = [pp.tile([P, NT], f32, tag="pp", name=f"po_{h}_{d}") for d in range(2)]
                psm = pp.tile([1, NT], f32, tag="pp")
                for kt in range(2 * CORES):
                    st, sp = kt == 0, kt == 2 * CORES - 1
                    nc.tensor.matmul(po[0][:], v_h[:, kt, 0:P], a_h[:, kt, :], start=st, stop=sp)
                    nc.tensor.matmul(po[1][:], v_h[:, kt, P:2 * P], a_h[:, kt, :], start=st, stop=sp)
                    nc.tensor.matmul(psm[:], ones_b[:], a_h[:, kt, :], start=st, stop=sp)
                rsum = stp.tile([1, NT], f32, tag="rsum")
                nc.vector.reciprocal(rsum[:], psm[:])
                prb = pp.tile([P, NT], f32, tag="pp")
                nc.tensor.matmul(prb[:], ones_f[:], rsum[:], start=True, stop=True)
                rbc = smp.tile([P, NT], f32, tag="rbc")
                nc.scalar.copy(rbc[:], prb[:])
                for d2 in range(2):
                    nc.vector.tensor_tensor(o_sb[:, 2 * h + d2, :], po[d2][:], rbc[:],
                                            ALU.mult)

            wo = load_w(wa, f'{prefix}_wo', H, H, tag="wa")
            bo = small(smp, f'{prefix}_bo', tag="bo")
            r_f = hf.tile([P, KS, NT], f32, tag="hf")
            def o_cons(mt, pt):
                nc.vector.scalar_tensor_tensor(r_f[:, mt, :], pt, bo[:, mt:mt + 1],
                                               src_f[:, mt, :], ALU.add, ALU.add)
            proj(wo, o_sb, KS, KS, o_cons)
            return r_f

        def ffn(n_f, n_b, prefix):
            """returns r = n_f + FFN(n_b). ff1/ff2 streamed as 2MB halves."""
            f1b_t = small(smp, f'{prefix}_ff1b', (P, FF // P), tag="f1b")
            h1 = ffp.tile([P, FF // P, NT], bf16, tag="ffh")
            for half in range(2):
                # ff1 columns [half*1024, (half+1)*1024) -> [128, 8, 1024]
                f1 = wa.tile([P, KS, H], bf16, tag="wa")
                nc.sync.dma_start(
                    f1[:], din[f'{prefix}_ff1']
                    .rearrange("(ks p) m -> p ks m", p=P)[:, :, half * H:(half + 1) * H])
                for m2 in range(KS):
                    mt = half * KS + m2
                    pt = pp.tile([P, NT], f32, tag="pp")
                    for ks in range(KS):
                        nc.tensor.matmul(pt[:], f1[:, ks, m2 * P:(m2 + 1) * P],
                                         n_b[:, ks, :], start=(ks == 0), stop=(ks == KS - 1))
                    nc.scalar.activation(h1[:, mt, :], pt[:], AF.Relu,
                                         bias=f1b_t[:, mt:mt + 1])
            # ff2: K = 2048 split into two row-halves, accumulated
            f2h = []
            for half in range(2):
                t = wa.tile([P, KS, H], bf16, tag="wa")
                nc.sync.dma_start(
                    t[:], din[f'{prefix}_ff2'][half * H:(half + 1) * H, :]
                    .rearrange("(ks p) m -> p ks m", p=P))
                f2h.append(t)
            f2b_t = small(smp, f'{prefix}_ff2b', tag="f2b")
            r_f = hf.tile([P, KS, NT], f32, tag="hf")
            for mt in range(KS):
                pt = pp.tile([P, NT], f32, tag="pp")
                for ks in range(FF // P):
                    nc.tensor.matmul(pt[:], f2h[ks // KS][:, ks % KS, mt * P:(mt + 1) * P],
                                     h1[:, ks, :], start=(ks == 0), stop=(ks == FF // P - 1))
                nc.vector.scalar_tensor_tensor(r_f[:, mt, :], pt[:], f2b_t[:, mt:mt + 1],
                                               n_f[:, mt, :], ALU.add, ALU.add)
            return r_f

        # ---------------- encoder ----------------
        mem_f, mem_b = cur_f, cur_b
        for i in range(2):
            k_loc, v_loc = kv_local(mem_b, f'enc{i}_sa')
            kv_out = kv_gather([(k_loc, v_loc)])
            r1 = attention(mem_f, mem_b, f'enc{i}_sa', kv_out, 0)
            n1_f, n1_b = layernorm(r1, f'enc{i}_ln1')
            r2 = ffn(n1_f, n1_b, f'enc{i}')
            mem_f, mem_b = layernorm(r2, f'enc{i}_ln2')
        mem_f, mem_b = layernorm(mem_f, 'enc_norm')

        if DEBUG:
            nc.sync.dma_start(dbg['dbg_mem'].rearrange("(ks p) t -> p ks t", p=P), mem_f[:])

        # cross-attention K/V for both decoder layers: gather once, early
        ca_outs = []
        for i in range(2):
            pair = kv_local(mem_b, f'dec{i}_ca')
            ca_outs.append(kv_gather([pair]))

        # ---------------- decoder ----------------
        t_f = cur_f
        t_b = hb.tile([P, KS, NT], bf16, tag="hb")
        nc.vector.tensor_copy(out=t_b[:], in_=t_f[:])
        for i in range(2):
            k_loc, v_loc = kv_local(t_b, f'dec{i}_sa')
            kv_out = kv_gather([(k_loc, v_loc)])
            r1 = attention(t_f, t_b, f'dec{i}_sa', kv_out, 0)
            n1_f, n1_b = layernorm(r1, f'dec{i}_ln1')
            r2 = attention(n1_f, n1_b, f'dec{i}_ca', ca_outs[i], 0)
            n2_f, n2_b = layernorm(r2, f'dec{i}_ln2')
            r3 = ffn(n2_f, n2_b, f'dec{i}')
            t_f, t_b = layernorm(r3, f'dec{i}_ln3')
        t_f, t_b = layernorm(t_f, 'dec_norm')

        # ---------------- heads ----------------
        wfc = load_w(wa, 'fc_w', H, H, tag="wa")
        bfc = small(smp, 'fc_b', tag="bfc")
        fc_b16 = qop.tile([P, KS, NT], bf16, tag="qbf")
        def fc_cons(mt, pt):
            nc.scalar.activation(fc_b16[:, mt, :], pt, AF.Identity,
                                 bias=bfc[:, mt:mt + 1])
        proj(wfc, t_b, KS, KS, fc_cons)

        wout = load_w(wa, 'out_w', H, OUT, tag="wa")
        bout = small(smp, 'out_b', tag="bout")
        y_f = hf.tile([P, KS, NT], f32, tag="hf")
        def out_cons(mt, pt):
            nc.scalar.activation(y_f[:, mt, :], pt, AF.Identity,
                                 bias=bout[:, mt:mt + 1])
        proj(wout, fc_b16, KS, KS, out_cons)

        nc.sync.dma_start(out_d.rearrange("(ks p) t -> p ks t", p=P), y_f[:, :, 1:NT])


_CACHE = {}


def _get_nc():
    if 'nc' not in _CACHE:
        nc = bacc.Bacc("TRN2", debug=False, num_devices=CORES)
        build(nc)
        nc.compile()
        _CACHE['nc'] = nc
    return _CACHE['nc']


def _run(inputs, trace=False):
    x = np.asarray(inputs['x'], np.float32)
    in_maps = _prep_inputs(x, inputs['params'])
    nc = _get_nc()
    res = run_bass_kernel_spmd(nc, in_maps, core_ids=list(range(CORES)), trace=trace)
    out = np.empty((S, OUT), np.float32)
    for c in range(CORES):
        out[c * SC:(c + 1) * SC] = res.results[c]['out'].T
    return out, res


def kernel(**inputs):
    out, _ = _run(inputs, trace=False)
    return out


def kernel_traced(**inputs):
    return _run(inputs, trace=True)
